# revision 24
# baseline (speedup 1.0000x reference)
"""Trainium2 Bass kernel for MultiHeadFrequencyCrossAttention.

Math note: the reference computes, per (batch, head) slice,
    energy = ifft2( fft2(Q) @ fft2(K)^T * dk ).real
Because the DFT matrix F satisfies F @ F^T = n * P (P = index-negation
permutation), this collapses EXACTLY to
    energy = dk * D * Q @ K~^T        with K~[j, d] = K[j, (-d) mod D]
i.e. plain attention with K's head-dim index flipped (mod D) and an extra
scale of dk * D = 512.  No FFTs are needed; the flip and scale are folded
into host-side slices of the Wk / Wq projection weights.

Sharding: 8 cores = 4 batches x 2 head-groups (4 heads each).  Each core
gets q[b]^T, kv[b]^T (pre-transposed on host so the contraction dim lands
on SBUF partitions) plus its slice of the projection weights, computes
attention for its 4 heads, and applies its slice of Wo.  The host sums the
two partial Wo products per batch (the unshard-reduce).

Precision scheme (PE fp32 matmuls are 4 cyc/row; fp16 is 1 cyc/row):
every value on the logit path is split hi/lo into two fp16 parts
(x = xh + xl, products of fp16 are exact in the fp32 PSUM accumulator), so
  x @ y ~= xh@yh + (xh@yl + xl@yh)     [~22-bit mantissa, err ~1e-6 rel]
One extra all-ones row in the stationary K operand times a "-rowmax" row
in the moving Q operand injects the softmax max-subtraction bias directly
into the S^T matmul.  The row max itself comes from a separate hi-only
fp16 pass (error ~ +-15 absolute on ~25000-scale logits, well inside the
exp() range window since A tiles are bf16).  A/V/output paths are plain
16-bit (error there stays relative, ~2e-3, no sharp-softmax blowup).

Per-core pipeline (T=1024, D=64):
  1. hi/lo projections -> per head: qm/km (fp16 hi + bias/ones row),
     qc/kc (fp16 [lo;hi] stacks for the cross matmul); vp t-major bf16
     with an all-ones column per head.
  2. max pass: S = qh @ kh^T per 128-row block (fp16), DVE reduce_max
     (negated) -> DRAM bounce -> fp16 "-rowmax" row of qm.
  3. main pass: S^T - max = cross(K=128) + main(K=65, w/ bias row)
     matmuls, ACT exp psum->sbuf bf16 directly in (j, i) layout.
  4. AV: A^T tiles are the moving operand; vp (with ones column) is
     stationary, accumulating [Y^T; rowsums] in one psum tensor.
  5. normalize Y^T by 1/rowsums (partition_broadcast + DVE mul), cast
     fp16, Wo partial product (fp16), DMA out.
"""

import numpy as np
from contextlib import ExitStack

import concourse.bass as bass
import concourse.tile as tile
from concourse import bacc, mybir
from concourse.bass_utils import run_bass_kernel_spmd

F32 = mybir.dt.float32
F16 = mybir.dt.float16
BF16 = mybir.dt.bfloat16
AX = mybir.AxisListType
AF = mybir.ActivationFunctionType

T = 1024          # sequence length
E = 512           # embed dim
H = 8             # total heads
D = E // H        # head dim = 64
NH = 4            # heads per core
DX = NH * (D + 1) # vp columns incl. ones = 260
N_CORES = 8
SCALE = float(D) * float(D) ** 0.5  # dk * D = 512.0

TRACE = False          # set by test harness; adds NTFF profiling
LAST_EXEC_NS = None


def _emit(ctx, tc, dram):
    nc = tc.nc
    const = ctx.enter_context(tc.tile_pool(name="const", bufs=1))
    ps_big = ctx.enter_context(tc.tile_pool(name="ps_big", bufs=2, space="PSUM"))
    ps_mix = ctx.enter_context(tc.tile_pool(name="ps_mix", bufs=2, space="PSUM"))
    atp = ctx.enter_context(tc.tile_pool(name="atp", bufs=6))
    outp = ctx.enter_context(tc.tile_pool(name="outp", bufs=8))
    dramp = ctx.enter_context(tc.tile_pool(name="dramp", bufs=1, space="DRAM"))

    # ---- input loads (all fp16 on the wire, one 3D DMA per matrix) ----
    def load1(name, cols):
        t3 = const.tile([128, 4, cols], F16, tag=name, name=name)
        nc.sync.dma_start(
            t3[:], dram[name][:].rearrange("(c p) t -> p c t", p=128)
        )
        return [t3[:, e, :] for e in range(4)]

    # load order matters: the first projection matmuls need wqh+ql first
    wqh = load1("wqh", NH * D)
    ql_in = load1("ql", T)
    wql = load1("wql", NH * D)
    qh_in = load1("qh", T)
    wkh = load1("wkh", NH * D)
    kvl_in = load1("kvl", T)
    wkl = load1("wkl", NH * D)
    kvh_in = load1("kvh", T)
    wv = load1("wv", DX)
    wo3 = const.tile([128, 2, E], F16, tag="wo", name="wo")
    nc.sync.dma_start(
        wo3[:], dram["wo"][:].rearrange("(g p) t -> p g t", p=128)
    )
    wo = [wo3[:, g, :] for g in range(2)]

    # PE warm-up: dummy matmuls fill the input-DMA window so the HAM clock
    # gate is already at 8/8 (2.4 GHz) when the projections start.
    wrm = const.tile([128, 512], F16, tag="wrm", name="wrm")
    nc.vector.memset(wrm[:], 0.0)
    for w in range(16):
        pw = ps_mix.tile([128, E], F32, tag="mix", name="psw")
        nc.tensor.matmul(pw[:], lhsT=wrm[:, 0:128], rhs=wrm[:],
                         start=True, stop=True)

    # ---- hi/lo projections ----
    # per head: qm (65, T) fp16 = [qp_hi; -rowmax(fp16) later]
    #           km (65, T) fp16 = [kp_hi; ones]
    #           qc (128, T) fp16 = [qp_lo; qp_hi]   (cross moving operand)
    #           kc (128, T) fp16 = [kp_hi; kp_lo]   (cross stationary)
    qm = [const.tile([65, T], F16, tag=f"qm{h}", name=f"qm{h}") for h in range(NH)]
    km = [const.tile([65, T], F16, tag=f"km{h}", name=f"km{h}") for h in range(NH)]
    qc = [const.tile([128, T], F16, tag=f"qc{h}", name=f"qc{h}") for h in range(NH)]
    kc = [const.tile([128, T], F16, tag=f"kc{h}", name=f"kc{h}") for h in range(NH)]

    for wh, wl, xh, xl, dm, dc, hi_row in (
        (wqh, wql, qh_in, ql_in, qm, qc, 64),   # qc rows: [lo; hi]
        (wkh, wkl, kvh_in, kvl_in, km, kc, 0),  # kc rows: [hi; lo]
    ):
        for m in range(2):  # head pair
            msl = slice(m * 128, (m + 1) * 128)
            ps = ps_big.tile([128, T], F32, tag="big", name="psb")
            for n in range(2):
                nsl = slice(n * 512, (n + 1) * 512)
                mms = (
                    # cross: Wh @ xl  +  Wl @ xh
                    [(wh[e], xl[e]) for e in range(4)]
                    + [(wl[e], xh[e]) for e in range(4)]
                    # main: Wh @ xh
                    + [(wh[e], xh[e]) for e in range(4)]
                )
                for i_mm, (lw, rx) in enumerate(mms):
                    nc.tensor.matmul(
                        ps[:, nsl],
                        lhsT=lw[:, msl],
                        rhs=rx[:, nsl],
                        start=(i_mm == 0), stop=(i_mm == len(mms) - 1),
                    )
            for hh in range(2):
                h = 2 * m + hh
                psl = slice(hh * 64, hh * 64 + 64)
                lo_row = 64 - hi_row
                # hi part (fp16 cast) into the K=65 "main" tile
                nc.scalar.copy(dm[h][0:64, :], ps[psl, :])
                # hi copy into the cross tile
                nc.vector.tensor_copy(dc[h][hi_row:hi_row + 64, :], dm[h][0:64, :])
                # lo part = ps - hi (fp16)
                nc.vector.tensor_sub(dc[h][lo_row:lo_row + 64, :], ps[psl, :],
                                     dm[h][0:64, :])
    for h in range(NH):
        nc.vector.memset(km[h][64:65, :], 1.0)

    # vp natural (t-major) + ones columns, bf16 (from fp16-hi inputs)
    vpx = [const.tile([128, DX], BF16, tag=f"vpx{t}", name=f"vpx{t}")
           for t in range(8)]
    for t in range(8):
        ps = ps_mix.tile([128, E], F32, tag="mix", name="pss")
        for e in range(4):
            nc.tensor.matmul(
                ps[:, 0:DX],
                lhsT=kvh_in[e][:, t * 128:(t + 1) * 128],
                rhs=wv[e][:],
                start=(e == 0), stop=(e == 3),
            )
        nc.scalar.copy(vpx[t][:], ps[:, 0:DX])
        for h4 in range(NH):
            c = h4 * (D + 1) + D
            nc.gpsimd.memset(vpx[t][:, c:c + 1], 1.0)

    # ---- per-head attention ----
    # Emission order software-pipelines heads: maxpass(0), maxpass(1),
    # main(0), maxpass(2), main(1), ... so the max-row DMA bounce and the
    # DVE reduce_max stream of head h+1 overlap head h's main-pass
    # matmuls, and PE never idles long enough to re-throttle (HAM).
    ypk = [const.tile([128, T], F32, tag=f"ypk{g}", name=f"ypk{g}")
           for g in range(2)]
    yun = [const.tile([64, T], F32, tag=f"yun{h}", name=f"yun{h}")
           for h in range(NH)]

    def maxpass(h):
        # max pass: S hi-only (fp16), row max per 128-row block
        colmax = const.tile([128, 8], F32, tag=f"cm{h}", name=f"cm{h}")
        for i in range(8):
            ps = ps_big.tile([128, T], F32, tag="big", name="psb")
            for n in range(2):
                nsl = slice(n * 512, (n + 1) * 512)
                nc.tensor.matmul(
                    ps[:, nsl],
                    lhsT=qm[h][0:64, i * 128:(i + 1) * 128],
                    rhs=km[h][0:64, nsl],
                    start=True, stop=True,
                )
            nc.vector.reduce_max(colmax[:, i:i + 1], ps[:], axis=AX.X,
                                 negate=True)
        # (128, 8) f32 -> (1, 1024) f32 row, via DRAM bounce
        sc = dramp.tile([8, 128], F32, tag=f"sc{h}", name=f"sc{h}")
        nc.sync.dma_start(sc[:].rearrange("c p -> p c"), colmax[:])
        mxf = const.tile([1, T], F32, tag=f"mx{h}", name=f"mx{h}")
        nc.sync.dma_start(mxf[:], sc[:].rearrange("c p -> (c p)"))
        nc.scalar.copy(qm[h][64:65, :], mxf[:])

    def mainpass(h):
        # main pass: S^T - max = cross + main(bias), exp, AV accumulate
        oex = ps_mix.tile([65, T], F32, tag="mix", name="oex")
        for j in range(8):
            jsl = slice(j * 128, (j + 1) * 128)
            ps = ps_big.tile([128, T], F32, tag="big", name="psb")
            for n in range(2):
                nsl = slice(n * 512, (n + 1) * 512)
                nc.tensor.matmul(
                    ps[:, nsl], lhsT=kc[h][:, jsl], rhs=qc[h][:, nsl],
                    start=True, stop=False,
                )
                nc.tensor.matmul(
                    ps[:, nsl], lhsT=km[h][:, jsl], rhs=qm[h][:, nsl],
                    start=False, stop=True,
                )
            at = atp.tile([128, T], BF16, tag="at", name="at")
            nc.scalar.activation(at[:], ps[:], AF.Exp)
            for n in range(2):
                nsl = slice(n * 512, (n + 1) * 512)
                nc.tensor.matmul(
                    oex[:, nsl],
                    lhsT=vpx[j][:, h * (D + 1):(h + 1) * (D + 1)],
                    rhs=at[:, nsl],
                    start=(j == 0), stop=(j == 7),
                )
        # Evacuate PSUM immediately (frees the oex slot for the next head);
        # the normalize chain below then runs off the critical path.
        nc.scalar.copy(yun[h][:], oex[0:64, :])
        # 1/sums: DVE reciprocal is ~8 cyc/elem, so run it in a (128, 8)
        # layout (DMA reshape through DRAM, off the critical path) instead
        # of 1024 elems on one lane.  (ACT Ln/Exp would mean table swaps.)
        sums = const.tile([1, T], F32, tag=f"sm{h}", name=f"sums{h}")
        nc.vector.tensor_copy(sums[:], oex[64:65, :])
        sd = dramp.tile([T], F32, tag=f"sd{h}", name=f"sd{h}")
        nc.sync.dma_start(sd[:], sums[:])
        s8 = const.tile([128, 8], F32, tag=f"s8{h}", name=f"s8{h}")
        nc.sync.dma_start(s8[:], sd[:].rearrange("(c p) -> p c", p=128))
        r8 = const.tile([128, 8], F32, tag=f"r8{h}", name=f"r8{h}")
        nc.vector.reciprocal(r8[:], s8[:])
        rd = dramp.tile([T], F32, tag=f"rd{h}", name=f"rd{h}")
        nc.sync.dma_start(rd[:].rearrange("(c p) -> p c", p=128), r8[:])
        recip = const.tile([1, T], F32, tag=f"rcp{h}", name=f"rcp{h}")
        nc.sync.dma_start(recip[:], rd[:])
        recb = const.tile([64, T], F32, tag=f"rcb{h}", name=f"rcb{h}")
        nc.gpsimd.partition_broadcast(recb[:], recip[:])
        g, half = divmod(h, 2)
        nc.vector.tensor_mul(
            ypk[g][half * 64:(half + 1) * 64, :], yun[h][:], recb[:]
        )

    # ---- output projection, split by head pair ----
    # g=0 (heads 0,1) runs mid-kernel right after ypk[0] completes; g=1
    # accumulates on top at the tail.  fp16 single: Y/Wo errors stay
    # relative (~5e-4), no sharp-softmax amplification.
    yh = [const.tile([128, T], F16, tag=f"yh{g}", name=f"yh{g}") for g in range(2)]
    ot = [outp.tile([128, E], F32, tag="ot", name=f"ot{i}") for i in range(8)]

    def wo_pass(g):
        nc.scalar.copy(yh[g][:], ypk[g][:])
        for i in range(8):
            pso = ps_mix.tile([128, E], F32, tag="mix", name="pso")
            nc.tensor.matmul(
                pso[:],
                lhsT=yh[g][:, i * 128:(i + 1) * 128],
                rhs=wo[g][:],
                start=True, stop=True,
            )
            if g == 0:
                nc.vector.tensor_copy(ot[i][:], pso[:])
            else:
                nc.vector.tensor_add(ot[i][:], ot[i][:], pso[:])
                nc.sync.dma_start(dram["out"][i * 128:(i + 1) * 128, :], ot[i][:])

    maxpass(0)
    maxpass(1)
    mainpass(0)
    maxpass(2)
    mainpass(1)
    maxpass(3)
    wo_pass(0)
    mainpass(2)
    mainpass(3)
    wo_pass(1)

def build_program():
    # Bacc (not raw Bass): its compile() splits multi-sem matmul waits onto
    # ldweights (TRN2 allows 1 wait/instruction), auto-inserts gpsimd
    # library loads for PartitionBroadcast, and lowers extended-ISA bytes.
    nc = bacc.Bacc("TRN2", target_bir_lowering=False, debug=False)
    dp = nc.declare_dram_parameter
    dram = {}
    for name in ("qh", "ql", "kvh", "kvl"):
        dram[name] = dp(name, [E, T], F16, isOutput=False)
    for name in ("wqh", "wql", "wkh", "wkl"):
        dram[name] = dp(name, [E, NH * D], F16, isOutput=False)
    dram["wv"] = dp("wv", [E, DX], F16, isOutput=False)
    dram["wo"] = dp("wo", [NH * D, E], F16, isOutput=False)
    dram["out"] = dp("out", [T, E], F32, isOutput=True)
    with ExitStack() as ctx:
        tc = ctx.enter_context(tile.TileContext(nc))
        _emit(ctx, tc, dram)
    nc.finalize()  # Bacc.finalize runs compile() then freezes
    return nc


_PROGRAM = None


def _get_program():
    global _PROGRAM
    if _PROGRAM is None:
        _PROGRAM = build_program()
    return _PROGRAM


def _split16(x):
    h = x.astype(np.float16)
    l = (x - h.astype(np.float32)).astype(np.float16)
    return h, l


def make_in_maps(q, kv, Wq, Wk, Wv, Wo):
    in_maps = []
    for c in range(N_CORES):
        b, g = divmod(c, 2)
        heads = [g * NH + j for j in range(NH)]
        idx_q = [d * H + h for h in heads for d in range(D)]
        idx_k = [((D - d) % D) * H + h for h in heads for d in range(D)]
        qTh, qTl = _split16(np.ascontiguousarray(q[b].T))
        kvTh, kvTl = _split16(np.ascontiguousarray(kv[b].T))
        wq_h, wq_l = _split16(Wq[:, idx_q] * np.float32(SCALE))
        wk_h, wk_l = _split16(Wk[:, idx_k])
        wv_c = np.zeros((E, DX), np.float16)
        for j, h in enumerate(heads):
            wv_c[:, j * (D + 1):j * (D + 1) + D] = \
                Wv[:, [d * H + h for d in range(D)]].astype(np.float16)
        in_maps.append({
            "qh": qTh, "ql": qTl, "kvh": kvTh, "kvl": kvTl,
            "wqh": wq_h, "wql": wq_l, "wkh": wk_h, "wkl": wk_l,
            "wv": wv_c,
            "wo": Wo[g * NH * D:(g + 1) * NH * D, :].astype(np.float16),
        })
    return in_maps


def kernel(**inputs):
    global LAST_EXEC_NS
    q = np.asarray(inputs["q"], dtype=np.float32)
    kv = np.asarray(inputs["kv"], dtype=np.float32)
    Wq = np.asarray(inputs["Wq"], dtype=np.float32)
    Wk = np.asarray(inputs["Wk"], dtype=np.float32)
    Wv = np.asarray(inputs["Wv"], dtype=np.float32)
    Wo = np.asarray(inputs["Wo"], dtype=np.float32)
    B = q.shape[0]

    nc = _get_program()
    in_maps = make_in_maps(q, kv, Wq, Wk, Wv, Wo)
    res = run_bass_kernel_spmd(nc, in_maps, list(range(N_CORES)), trace=TRACE)
    LAST_EXEC_NS = res.exec_time_ns

    out = np.empty((B, T, E), np.float32)
    for b in range(B):
        out[b] = res.results[2 * b]["out"] + res.results[2 * b + 1]["out"]
    return out
